# revision 17
# baseline (speedup 1.0000x reference)
"""Trainium2 Bass kernel for nn_Attention (B=4, N=2048, DIM=1024, 16 heads x 64).

Sharding: 8 cores = 4 batches x 2 head-groups. Core c handles batch c//2 and
heads [8*(c%2), 8*(c%2)+8). Each core computes QKV projection, attention and
the output projection for its (batch, head-group); the two cores sharing a
batch produce partial output projections that the host sums (+ bias).

Device-side layout (per core):
  x^T   [1024, 2048]  bf16  (k on partitions; built via PE transposes)
  qkT   [1024, 2048]  bf16  Q^T (c-tiles 0-3) / K^T (c-tiles 4-7); within a
                            c-tile, partitions 0-63 = even head, 64-127 = odd
  vprime [2048, 520]  bf16  V with a ones column per head (65-stride groups)
  S^T = K^T.T @ Q^T per head via K=64 row-tiled matmuls; both heads of a
      pair-group stacked into one 2-bank PSUM tile [128, 1024]
  P^T = exp(SCALE * S^T) on ACT, one [128, 1024] activation per j-tile
  AV: out[i, 0:65 | 65:130] = P^T.T @ [V | 1] for the two heads, accumulated
      over j-tiles in one PSUM bank (denominators at cols 64 / 129),
      normalized with tensor_scalar(mult, recip) per partition
  attn^T via PE transpose -> proj^T = W_out.T-tiled matmuls -> DRAM [1024, 2048]

The Q/K projections for pair-group pg+1 are emitted interleaved with the
attention of pair-group pg so the PE instruction stream stays dense during the
ACT-bound softmax stretches (keeps the HAM clock at 2.4 GHz).
"""

import sys

if "/opt/trn_rl_repo" not in sys.path:
    sys.path.insert(0, "/opt/trn_rl_repo")

import numpy as np
import ml_dtypes

import concourse.bass as bass
import concourse.mybir as mybir
from concourse.tile import TileContext
from concourse.bass_utils import run_bass_kernel_spmd

P = 128
B, N, DIM = 4, 2048, 1024
HEADS, DIM_HEAD = 16, 64
INNER = HEADS * DIM_HEAD
SCALE = DIM_HEAD**-0.5
LOCAL_HEADS = 8          # heads per core
LOCAL_INNER = LOCAL_HEADS * DIM_HEAD      # 512
QKV_COLS = 3 * LOCAL_INNER                # 1536
TT = N // P              # 16 token tiles
KC = DIM // P            # 8 contraction chunks
IC = N // 512            # 4 query chunks of 512
JT = N // P              # 16 key tiles
PG = LOCAL_HEADS // 2    # 4 head pair-groups
NT = DIM // P            # 8 output col tiles
KT = LOCAL_INNER // P    # 4 proj contraction chunks

BF16 = mybir.dt.bfloat16
F32 = mybir.dt.float32
EXP = mybir.ActivationFunctionType.Exp


def _split_multi_waits(nc):
    """This env's walrus encodes at most ONE sync-wait per instruction; hoist
    extras onto InstEventSemaphore carriers inserted just before, same engine."""
    f = nc.m.functions[0]
    rebuilt = []
    for blk in f.blocks:
        newlist = []
        for inst in blk.instructions:
            si = inst.sync_info
            if si is not None and len(si.on_wait) > 1:
                waits = list(si.on_wait)
                eng = inst.engine
                for w in waits[:-1]:
                    ev = mybir.InstEventSemaphore(
                        name=nc.get_next_instruction_name(), ins=[], outs=[])
                    ev.engine = eng
                    ev.sync_info = mybir.SyncInfo(on_wait=[w], on_update=[])
                    newlist.append(ev)
                inst.sync_info = mybir.SyncInfo(
                    on_wait=[waits[-1]], on_update=list(si.on_update))
            newlist.append(inst)
        rebuilt.append((blk, newlist))
    for blk, newlist in rebuilt:
        blk.instructions = newlist
    return nc


def build_attention_nc():
    nc = bass.Bass(trn_type="TRN2", num_devices=8)
    x_d = nc.dram_tensor("x", [DIM, N], BF16, kind="ExternalInput")
    wqkv_d = nc.dram_tensor("wqkv", [DIM, QKV_COLS], BF16, kind="ExternalInput")
    wout_d = nc.dram_tensor("wout", [LOCAL_INNER, DIM], BF16, kind="ExternalInput")
    ident_d = nc.dram_tensor("ident", [P, P], BF16, kind="ExternalInput")
    o_d = nc.dram_tensor("o", [DIM, N], F32, kind="ExternalOutput")

    with TileContext(nc, num_cores=8) as tc:
        with tc.tile_pool(name="persist", bufs=1) as persist:
            ident = persist.tile([P, P], BF16, tag="ident")
            nc.sync.dma_start(ident[:], ident_d[:])
            xT = [persist.tile([P, N], BF16, name=f"xT{k}", tag=f"xT{k}")
                  for k in range(KC)]
            wqkv = [persist.tile([P, QKV_COLS], BF16, name=f"wqkv{k}", tag=f"wqkv{k}")
                    for k in range(KC)]
            qkT = [persist.tile([P, N], BF16, name=f"qkT{c}", tag=f"qkT{c}")
                   for c in range(2 * KT)]
            vprime = [persist.tile([P, 65 * LOCAL_HEADS], BF16, name=f"vp{t}",
                                   tag=f"vp{t}") for t in range(TT)]
            wout = [persist.tile([P, DIM], BF16, name=f"wout{k}", tag=f"wout{k}")
                    for k in range(KT)]

            for k in range(KC):
                nc.sync.dma_start(wqkv[k][:], wqkv_d[k * P:(k + 1) * P, :])

            # ---- Stage A+B: load x^T (host-transposed), compute V' and pg0 Q/K ----
            for k in range(KC):
                nc.sync.dma_start(xT[k][:, 0:1024], x_d[k * P:(k + 1) * P, 0:1024])
            for k in range(KC):
                nc.sync.dma_start(xT[k][:, 1024:2048],
                                  x_d[k * P:(k + 1) * P, 1024:2048])
            for k in range(KT):
                nc.sync.dma_start(wout[k][:], wout_d[k * P:(k + 1) * P, :])
            with tc.tile_pool(name="psBv", bufs=4, space="PSUM") as psBv:
                for ic in range(IC):
                    for ct in (0, KT):
                        ps = psBv.tile([P, 512], F32, name=f"qk0_{ct}_{ic}", tag="v")
                        for k in range(KC):
                            nc.tensor.matmul(
                                ps[:], lhsT=wqkv[k][:, ct * P:(ct + 1) * P],
                                rhs=xT[k][:, ic * 512:(ic + 1) * 512],
                                start=(k == 0), stop=(k == KC - 1))
                        nc.vector.tensor_copy(
                            qkT[ct][:, ic * 512:(ic + 1) * 512], ps[:])

            # ---- Stages C+D: pipelined attention + projection ----
            # Per (pg, ic): 16 j-tiles of S^T (both heads stacked in one
            # [128,1024] 2-bank PSUM tile) feed ACT exp. The previous
            # iteration's AV matmuls are interleaved one 8-MM quantum per
            # j-tile so the PE stream stays dense while ACT chews. Q/K
            # projections for pg+1 (or output projections, during the last
            # pg) slot in as additional filler.
            with tc.tile_pool(name="pT", bufs=28) as pT_pool, \
                 tc.tile_pool(name="psS", bufs=2, space="PSUM") as psS, \
                 tc.tile_pool(name="psF", bufs=2, space="PSUM") as psF, \
                 tc.tile_pool(name="psAV", bufs=2, space="PSUM") as psAV, \
                 tc.tile_pool(name="smallsb", bufs=8) as smallsb, \
                 tc.tile_pool(name="osb", bufs=3) as osb_pool:
                attnT_t = {}
                for kt in range(KT):
                    for ic in range(IC):
                        attnT_t[(kt, ic)] = persist.tile(
                            [P, 512], BF16, name=f"attnT_{kt}_{ic}",
                            tag=f"attnT_{kt}_{ic}")

                def v_run(tt, _unused):
                    nc.vector.memset(vprime[tt][:], 1.0)
                    ps = psF.tile([P, 512], F32, name=f"v_{tt}", tag="f")
                    for k in range(KC):
                        nc.tensor.matmul(
                            ps[:], lhsT=xT[k][:, tt * P:(tt + 1) * P],
                            rhs=wqkv[k][:, 2 * LOCAL_INNER:3 * LOCAL_INNER],
                            start=(k == 0), stop=(k == KC - 1))
                    for h in range(LOCAL_HEADS):
                        nc.vector.tensor_copy(
                            vprime[tt][:, h * 65:h * 65 + 64],
                            ps[:, h * 64:(h + 1) * 64])

                def qk_run(ct, ic):
                    ps = psF.tile([P, 512], F32, name=f"qk_{ct}_{ic}", tag="f")
                    for k in range(KC):
                        nc.tensor.matmul(
                            ps[:], lhsT=wqkv[k][:, ct * P:(ct + 1) * P],
                            rhs=xT[k][:, ic * 512:(ic + 1) * 512],
                            start=(k == 0), stop=(k == KC - 1))
                    nc.vector.tensor_copy(qkT[ct][:, ic * 512:(ic + 1) * 512], ps[:])

                def proj_run(nt, ic):
                    ps = psF.tile([P, 512], F32, name=f"proj_{nt}_{ic}", tag="f")
                    for kt in range(KT):
                        nc.tensor.matmul(
                            ps[:], lhsT=wout[kt][:, nt * P:(nt + 1) * P],
                            rhs=attnT_t[(kt, ic)][:],
                            start=(kt == 0), stop=(kt == KT - 1))
                    osb = osb_pool.tile([P, 512], F32, name=f"osb_{nt}_{ic}",
                                        tag="osb")
                    nc.vector.tensor_copy(osb[:], ps[:])
                    nc.sync.dma_start(
                        o_d[nt * P:(nt + 1) * P, ic * 512:(ic + 1) * 512], osb[:])

                def av_gen(pg, ic, pT):
                    # AV + normalize + transpose for one (pg, ic), in 8-MM quanta
                    for it in range(4):
                        av = psAV.tile([P, 130], F32, name=f"av_{pg}_{ic}_{it}",
                                       tag="av")
                        for half in range(2):
                            base = 512 * half
                            voff = (2 * pg + half) * 65
                            for j0 in range(0, JT, 8):
                                for jt in range(j0, j0 + 8):
                                    nc.tensor.matmul(
                                        av[:, 65 * half:65 * half + 65],
                                        lhsT=pT[jt][:, base + it * P:
                                                    base + (it + 1) * P],
                                        rhs=vprime[jt][:, voff:voff + 65],
                                        start=(jt == 0), stop=(jt == JT - 1))
                                yield
                        for half in range(2):
                            recip = smallsb.tile([P, 1], F32, tag="recip")
                            nc.vector.reciprocal(
                                recip[:], av[:, 65 * half + 64:65 * half + 65])
                            attn = smallsb.tile([P, 64], BF16, tag="attn")
                            nc.vector.tensor_scalar_mul(
                                attn[:], av[:, 65 * half:65 * half + 64], recip[:])
                            tp = psF.tile([64, P], BF16,
                                          name=f"tp_{pg}_{ic}_{it}_{half}",
                                          tag="f")
                            nc.tensor.transpose(tp[:], attn[:], ident[:])
                            nc.vector.tensor_copy(
                                attnT_t[(pg, ic)][half * 64:(half + 1) * 64,
                                                  it * P:(it + 1) * P], tp[:])
                        yield

                def advance(gen, n):
                    for _ in range(n):
                        if next(gen, "done") == "done":
                            return None
                    return gen

                backlog = None
                for pg in range(PG):
                    kt_t = qkT[KT + pg]
                    qt_t = qkT[pg]
                    if pg + 1 < PG:
                        filler = [(qk_run, pg + 1, ic) for ic in range(IC)] + \
                                 [(qk_run, KT + pg + 1, ic) for ic in range(IC)]
                        slots = (5, 11)
                        if pg == 0:
                            filler = [(v_run, tt, 0) for tt in range(TT)] + filler
                            slots = tuple(range(16))
                    else:
                        # last pg: output projections of completed ic slices
                        filler = [(proj_run, nt, ic) for ic in range(IC - 2)
                                  for nt in range(NT)]
                        slots = (1, 3, 5, 7, 9, 11, 13, 15)
                    for ic in range(IC):
                        i0 = ic * 512
                        pT = []
                        for jt in range(JT):
                            ps = psS.tile([P, 1024], F32)
                            nc.tensor.matmul(
                                ps[:, 0:512],
                                lhsT=kt_t[0:64, jt * P:(jt + 1) * P],
                                rhs=qt_t[0:64, i0:i0 + 512])
                            nc.tensor.matmul(
                                ps[:, 512:1024],
                                lhsT=kt_t[64:128, jt * P:(jt + 1) * P],
                                rhs=qt_t[64:128, i0:i0 + 512])
                            pt = pT_pool.tile([P, 1024], BF16)
                            nc.scalar.activation(pt[:], ps[:], EXP, scale=SCALE)
                            pT.append(pt)
                            if backlog is not None:
                                backlog = advance(backlog, 1)
                            if jt in slots and filler and (pg < PG - 1 or ic >= 2):
                                fn, a, b = filler.pop(0)
                                fn(a, b)
                        while backlog is not None:
                            backlog = advance(backlog, 4)
                        backlog = av_gen(pg, ic, pT)
                tailq = [(nt, IC - 2) for nt in range(NT)]
                while backlog is not None:
                    backlog = advance(backlog, 4)
                    if tailq:
                        nt, ic2 = tailq.pop(0)
                        proj_run(nt, ic2)
                for nt, ic2 in tailq:
                    proj_run(nt, ic2)
                for nt in range(NT):
                    proj_run(nt, IC - 1)

    _split_multi_waits(nc)
    return nc


_NC_CACHE = {}


def _get_nc():
    if "nc" not in _NC_CACHE:
        _NC_CACHE["nc"] = build_attention_nc()
    return _NC_CACHE["nc"]


def make_in_maps(x, w_qkv, w_out):
    bf = ml_dtypes.bfloat16
    ident = np.eye(P, dtype=bf)
    in_maps = []
    for c in range(8):
        b, g = c // 2, c % 2
        lo = LOCAL_INNER * g
        wq = w_qkv[:, lo:lo + LOCAL_INNER]
        wk = w_qkv[:, INNER + lo:INNER + lo + LOCAL_INNER]
        wv = w_qkv[:, 2 * INNER + lo:2 * INNER + lo + LOCAL_INNER]
        in_maps.append({
            "x": np.ascontiguousarray(x[b].T).astype(bf),
            "wqkv": np.ascontiguousarray(
                np.concatenate([wq, wk, wv], axis=1)).astype(bf),
            "wout": np.ascontiguousarray(
                w_out[lo:lo + LOCAL_INNER, :]).astype(bf),
            "ident": ident,
        })
    return in_maps


def combine_outputs(results, b_out):
    out = np.empty((B, N, DIM), dtype=np.float32)
    for b in range(B):
        acc = results[2 * b]["o"] + results[2 * b + 1]["o"]
        out[b] = acc.T + b_out[None, :]
    return out


def kernel(x, w_qkv, w_out, b_out, _trace=False):
    x = np.asarray(x, dtype=np.float32)
    w_qkv = np.asarray(w_qkv, dtype=np.float32)
    w_out = np.asarray(w_out, dtype=np.float32)
    b_out = np.asarray(b_out, dtype=np.float32)
    nc = _get_nc()
    in_maps = make_in_maps(x, w_qkv, w_out)
    res = run_bass_kernel_spmd(nc, in_maps, core_ids=list(range(8)), trace=_trace)
    out = combine_outputs(res.results, b_out)
    if _trace:
        return out, res
    return out


# revision 18
# speedup vs baseline: 1.0160x; 1.0160x over previous
"""Trainium2 Bass kernel for nn_Attention (B=4, N=2048, DIM=1024, 16 heads x 64).

Sharding: 8 cores = 4 batches x 2 head-groups. Core c handles batch c//2 and
heads [8*(c%2), 8*(c%2)+8). Each core computes QKV projection, attention and
the output projection for its (batch, head-group); the two cores sharing a
batch produce partial output projections that the host sums (+ bias).

Device-side layout (per core):
  x^T   [1024, 2048]  bf16  (k on partitions; built via PE transposes)
  qkT   [1024, 2048]  bf16  Q^T (c-tiles 0-3) / K^T (c-tiles 4-7); within a
                            c-tile, partitions 0-63 = even head, 64-127 = odd
  vprime [2048, 520]  bf16  V with a ones column per head (65-stride groups)
  S^T = K^T.T @ Q^T per head via K=64 row-tiled matmuls; both heads of a
      pair-group stacked into one 2-bank PSUM tile [128, 1024]
  P^T = exp(SCALE * S^T) on ACT, one [128, 1024] activation per j-tile
  AV: out[i, 0:65 | 65:130] = P^T.T @ [V | 1] for the two heads, accumulated
      over j-tiles in one PSUM bank (denominators at cols 64 / 129),
      normalized with tensor_scalar(mult, recip) per partition
  attn^T via PE transpose -> proj^T = W_out.T-tiled matmuls -> DRAM [1024, 2048]

The Q/K projections for pair-group pg+1 are emitted interleaved with the
attention of pair-group pg so the PE instruction stream stays dense during the
ACT-bound softmax stretches (keeps the HAM clock at 2.4 GHz).
"""

import sys

if "/opt/trn_rl_repo" not in sys.path:
    sys.path.insert(0, "/opt/trn_rl_repo")

import numpy as np
import ml_dtypes

import concourse.bass as bass
import concourse.mybir as mybir
from concourse.tile import TileContext
from concourse.bass_utils import run_bass_kernel_spmd

P = 128
B, N, DIM = 4, 2048, 1024
HEADS, DIM_HEAD = 16, 64
INNER = HEADS * DIM_HEAD
SCALE = DIM_HEAD**-0.5
LOCAL_HEADS = 8          # heads per core
LOCAL_INNER = LOCAL_HEADS * DIM_HEAD      # 512
QKV_COLS = 3 * LOCAL_INNER                # 1536
TT = N // P              # 16 token tiles
KC = DIM // P            # 8 contraction chunks
IC = N // 512            # 4 query chunks of 512
JT = N // P              # 16 key tiles
PG = LOCAL_HEADS // 2    # 4 head pair-groups
NT = DIM // P            # 8 output col tiles
KT = LOCAL_INNER // P    # 4 proj contraction chunks

BF16 = mybir.dt.bfloat16
F32 = mybir.dt.float32
EXP = mybir.ActivationFunctionType.Exp


def _split_multi_waits(nc):
    """This env's walrus encodes at most ONE sync-wait per instruction; hoist
    extras onto InstEventSemaphore carriers inserted just before, same engine."""
    f = nc.m.functions[0]
    rebuilt = []
    for blk in f.blocks:
        newlist = []
        for inst in blk.instructions:
            si = inst.sync_info
            if si is not None and len(si.on_wait) > 1:
                waits = list(si.on_wait)
                eng = inst.engine
                for w in waits[:-1]:
                    ev = mybir.InstEventSemaphore(
                        name=nc.get_next_instruction_name(), ins=[], outs=[])
                    ev.engine = eng
                    ev.sync_info = mybir.SyncInfo(on_wait=[w], on_update=[])
                    newlist.append(ev)
                inst.sync_info = mybir.SyncInfo(
                    on_wait=[waits[-1]], on_update=list(si.on_update))
            newlist.append(inst)
        rebuilt.append((blk, newlist))
    for blk, newlist in rebuilt:
        blk.instructions = newlist
    return nc


def build_attention_nc():
    nc = bass.Bass(trn_type="TRN2", num_devices=8)
    x_d = nc.dram_tensor("x", [DIM, N], BF16, kind="ExternalInput")
    wqkv_d = nc.dram_tensor("wqkv", [DIM, QKV_COLS], BF16, kind="ExternalInput")
    wout_d = nc.dram_tensor("wout", [LOCAL_INNER, DIM], BF16, kind="ExternalInput")
    ident_d = nc.dram_tensor("ident", [P, P], BF16, kind="ExternalInput")
    o_d = nc.dram_tensor("o", [DIM, N], F32, kind="ExternalOutput")

    with TileContext(nc, num_cores=8) as tc:
        with tc.tile_pool(name="persist", bufs=1) as persist:
            ident = persist.tile([P, P], BF16, tag="ident")
            nc.sync.dma_start(ident[:], ident_d[:])
            xT = [persist.tile([P, N], BF16, name=f"xT{k}", tag=f"xT{k}")
                  for k in range(KC)]
            wqkv = [persist.tile([P, QKV_COLS], BF16, name=f"wqkv{k}", tag=f"wqkv{k}")
                    for k in range(KC)]
            qkT = [persist.tile([P, N], BF16, name=f"qkT{c}", tag=f"qkT{c}")
                   for c in range(2 * KT)]
            vprime = [persist.tile([P, 65 * LOCAL_HEADS], BF16, name=f"vp{t}",
                                   tag=f"vp{t}") for t in range(TT)]
            wout = [persist.tile([P, DIM], BF16, name=f"wout{k}", tag=f"wout{k}")
                    for k in range(KT)]

            for k in range(KC):
                nc.sync.dma_start(wqkv[k][:], wqkv_d[k * P:(k + 1) * P, :])

            # ---- Stage A+B: load x^T (host-transposed), compute V' and pg0 Q/K ----
            for k in range(KC):
                nc.sync.dma_start(xT[k][:, 0:1024], x_d[k * P:(k + 1) * P, 0:1024])
            for k in range(KC):
                nc.sync.dma_start(xT[k][:, 1024:2048],
                                  x_d[k * P:(k + 1) * P, 1024:2048])
            for k in range(KT):
                nc.sync.dma_start(wout[k][:], wout_d[k * P:(k + 1) * P, :])
            with tc.tile_pool(name="psBv", bufs=4, space="PSUM") as psBv:
                for tt in range(TT):
                    nc.vector.memset(vprime[tt][:], 1.0)
                    ps = psBv.tile([P, 512], F32, name=f"v_{tt}", tag="v")
                    for k in range(KC):
                        nc.tensor.matmul(
                            ps[:], lhsT=xT[k][:, tt * P:(tt + 1) * P],
                            rhs=wqkv[k][:, 2 * LOCAL_INNER:3 * LOCAL_INNER],
                            start=(k == 0), stop=(k == KC - 1))
                    for h in range(LOCAL_HEADS):
                        nc.vector.tensor_copy(
                            vprime[tt][:, h * 65:h * 65 + 64],
                            ps[:, h * 64:(h + 1) * 64])
                for ic in range(IC):
                    for ct in (0, KT):
                        ps = psBv.tile([P, 512], F32, name=f"qk0_{ct}_{ic}", tag="v")
                        for k in range(KC):
                            nc.tensor.matmul(
                                ps[:], lhsT=wqkv[k][:, ct * P:(ct + 1) * P],
                                rhs=xT[k][:, ic * 512:(ic + 1) * 512],
                                start=(k == 0), stop=(k == KC - 1))
                        nc.vector.tensor_copy(
                            qkT[ct][:, ic * 512:(ic + 1) * 512], ps[:])

            # ---- Stages C+D: pipelined attention + projection ----
            # Per (pg, ic): 16 j-tiles of S^T (both heads stacked in one
            # [128,1024] 2-bank PSUM tile) feed ACT exp. The previous
            # iteration's AV matmuls are interleaved one 8-MM quantum per
            # j-tile so the PE stream stays dense while ACT chews. Q/K
            # projections for pg+1 (or output projections, during the last
            # pg) slot in as additional filler.
            with tc.tile_pool(name="pT", bufs=28) as pT_pool, \
                 tc.tile_pool(name="psS", bufs=2, space="PSUM") as psS, \
                 tc.tile_pool(name="psF", bufs=2, space="PSUM") as psF, \
                 tc.tile_pool(name="psAV", bufs=2, space="PSUM") as psAV, \
                 tc.tile_pool(name="smallsb", bufs=8) as smallsb, \
                 tc.tile_pool(name="osb", bufs=3) as osb_pool:
                attnT_t = {}
                for kt in range(KT):
                    for ic in range(IC):
                        attnT_t[(kt, ic)] = persist.tile(
                            [P, 512], BF16, name=f"attnT_{kt}_{ic}",
                            tag=f"attnT_{kt}_{ic}")

                def qk_run(ct, ic):
                    ps = psF.tile([P, 512], F32, name=f"qk_{ct}_{ic}", tag="f")
                    for k in range(KC):
                        nc.tensor.matmul(
                            ps[:], lhsT=wqkv[k][:, ct * P:(ct + 1) * P],
                            rhs=xT[k][:, ic * 512:(ic + 1) * 512],
                            start=(k == 0), stop=(k == KC - 1))
                    nc.vector.tensor_copy(qkT[ct][:, ic * 512:(ic + 1) * 512], ps[:])

                def proj_run(nt, ic):
                    ps = psF.tile([P, 512], F32, name=f"proj_{nt}_{ic}", tag="f")
                    for kt in range(KT):
                        nc.tensor.matmul(
                            ps[:], lhsT=wout[kt][:, nt * P:(nt + 1) * P],
                            rhs=attnT_t[(kt, ic)][:],
                            start=(kt == 0), stop=(kt == KT - 1))
                    osb = osb_pool.tile([P, 512], F32, name=f"osb_{nt}_{ic}",
                                        tag="osb")
                    nc.vector.tensor_copy(osb[:], ps[:])
                    nc.sync.dma_start(
                        o_d[nt * P:(nt + 1) * P, ic * 512:(ic + 1) * 512], osb[:])

                def av_gen(pg, ic, pT):
                    # AV + normalize + transpose for one (pg, ic), in 8-MM quanta
                    for it in range(4):
                        av = psAV.tile([P, 130], F32, name=f"av_{pg}_{ic}_{it}",
                                       tag="av")
                        for half in range(2):
                            base = 512 * half
                            voff = (2 * pg + half) * 65
                            for j0 in range(0, JT, 8):
                                for jt in range(j0, j0 + 8):
                                    nc.tensor.matmul(
                                        av[:, 65 * half:65 * half + 65],
                                        lhsT=pT[jt][:, base + it * P:
                                                    base + (it + 1) * P],
                                        rhs=vprime[jt][:, voff:voff + 65],
                                        start=(jt == 0), stop=(jt == JT - 1))
                                yield
                        for half in range(2):
                            recip = smallsb.tile([P, 1], F32, tag="recip")
                            nc.vector.reciprocal(
                                recip[:], av[:, 65 * half + 64:65 * half + 65])
                            attn = smallsb.tile([P, 64], BF16, tag="attn")
                            nc.vector.tensor_scalar_mul(
                                attn[:], av[:, 65 * half:65 * half + 64], recip[:])
                            tp = psF.tile([64, P], BF16,
                                          name=f"tp_{pg}_{ic}_{it}_{half}",
                                          tag="f")
                            nc.tensor.transpose(tp[:], attn[:], ident[:])
                            nc.vector.tensor_copy(
                                attnT_t[(pg, ic)][half * 64:(half + 1) * 64,
                                                  it * P:(it + 1) * P], tp[:])
                        yield

                def advance(gen, n):
                    for _ in range(n):
                        if next(gen, "done") == "done":
                            return None
                    return gen

                backlog = None
                for pg in range(PG):
                    kt_t = qkT[KT + pg]
                    qt_t = qkT[pg]
                    if pg + 1 < PG:
                        filler = [(qk_run, pg + 1, ic) for ic in range(IC)] + \
                                 [(qk_run, KT + pg + 1, ic) for ic in range(IC)]
                        slots = (5, 11)
                    else:
                        # last pg: output projections of completed ic slices
                        filler = [(proj_run, nt, ic) for ic in range(IC - 2)
                                  for nt in range(NT)]
                        slots = (1, 3, 5, 7, 9, 11, 13, 15)
                    for ic in range(IC):
                        i0 = ic * 512
                        pT = []
                        for jt in range(JT):
                            ps = psS.tile([P, 1024], F32)
                            nc.tensor.matmul(
                                ps[:, 0:512],
                                lhsT=kt_t[0:64, jt * P:(jt + 1) * P],
                                rhs=qt_t[0:64, i0:i0 + 512])
                            nc.tensor.matmul(
                                ps[:, 512:1024],
                                lhsT=kt_t[64:128, jt * P:(jt + 1) * P],
                                rhs=qt_t[64:128, i0:i0 + 512])
                            pt = pT_pool.tile([P, 1024], BF16)
                            nc.scalar.activation(pt[:], ps[:], EXP, scale=SCALE)
                            pT.append(pt)
                            if backlog is not None:
                                backlog = advance(backlog, 1)
                            if jt in slots and filler and (pg < PG - 1 or ic >= 2):
                                fn, a, b = filler.pop(0)
                                fn(a, b)
                        while backlog is not None:
                            backlog = advance(backlog, 4)
                        backlog = av_gen(pg, ic, pT)
                tailq = [(nt, IC - 2) for nt in range(NT)]
                while backlog is not None:
                    backlog = advance(backlog, 4)
                    if tailq:
                        nt, ic2 = tailq.pop(0)
                        proj_run(nt, ic2)
                for nt, ic2 in tailq:
                    proj_run(nt, ic2)
                for nt in range(NT):
                    proj_run(nt, IC - 1)

    _split_multi_waits(nc)
    return nc


_NC_CACHE = {}


def _get_nc():
    if "nc" not in _NC_CACHE:
        _NC_CACHE["nc"] = build_attention_nc()
    return _NC_CACHE["nc"]


def make_in_maps(x, w_qkv, w_out):
    bf = ml_dtypes.bfloat16
    ident = np.eye(P, dtype=bf)
    in_maps = []
    for c in range(8):
        b, g = c // 2, c % 2
        lo = LOCAL_INNER * g
        wq = w_qkv[:, lo:lo + LOCAL_INNER]
        wk = w_qkv[:, INNER + lo:INNER + lo + LOCAL_INNER]
        wv = w_qkv[:, 2 * INNER + lo:2 * INNER + lo + LOCAL_INNER]
        in_maps.append({
            "x": np.ascontiguousarray(x[b].T).astype(bf),
            "wqkv": np.ascontiguousarray(
                np.concatenate([wq, wk, wv], axis=1)).astype(bf),
            "wout": np.ascontiguousarray(
                w_out[lo:lo + LOCAL_INNER, :]).astype(bf),
            "ident": ident,
        })
    return in_maps


def combine_outputs(results, b_out):
    out = np.empty((B, N, DIM), dtype=np.float32)
    for b in range(B):
        acc = results[2 * b]["o"] + results[2 * b + 1]["o"]
        out[b] = acc.T + b_out[None, :]
    return out


def kernel(x, w_qkv, w_out, b_out, _trace=False):
    x = np.asarray(x, dtype=np.float32)
    w_qkv = np.asarray(w_qkv, dtype=np.float32)
    w_out = np.asarray(w_out, dtype=np.float32)
    b_out = np.asarray(b_out, dtype=np.float32)
    nc = _get_nc()
    in_maps = make_in_maps(x, w_qkv, w_out)
    res = run_bass_kernel_spmd(nc, in_maps, core_ids=list(range(8)), trace=_trace)
    out = combine_outputs(res.results, b_out)
    if _trace:
        return out, res
    return out
